# revision 13
# baseline (speedup 1.0000x reference)
"""Trainium2 Bass kernel for cross multi-head attention (B=4, N=1024, D=1024, H=16).

Sharding: 8 cores = 4 batches x 2 head-groups (8 heads each), Megatron-style.
Each core computes, for its (batch b, head-group g):
  Q^T, K^T = W[hs] @ x^T / l^T      (projections, transposed layout)
  S^T_h    = K_h @ Q_h^T            (scores, [nk, nq] layout)
  E        = exp(S^T * scale)       (unnormalized; softmax max-shift is
                                     unnecessary: |scores| <~ 1)
  O'^T     = V'^T E  with V' = [V | 1]  -> rows 0:64 = unnorm AV, row 64 = Z
  O^T      = O'^T[0:64] * (1/Z)     (normalized attention output, transposed)
  outp     = O^T.T @ Wo[:, hs].T    (partial out-projection)
Host side: gathers per-core outputs, transposes/normalizes att and out_1,
sums the two head-group partials of out, adds biases that commute
(Wo bias). All matmuls run as float32r (tf32-like, ~1.5e-4 rel err).
"""

import sys

if "/opt/trn_rl_repo" not in sys.path:
    sys.path.insert(0, "/opt/trn_rl_repo")

import numpy as np

import concourse.bacc as bacc
import concourse.mybir as mybir
import concourse.tile as tile
from concourse.bass_utils import run_bass_kernel_spmd

F32 = mybir.dt.float32
F32R = mybir.dt.float32r
AF = mybir.ActivationFunctionType
OP = mybir.AluOpType

B, N, D, H = 4, 1024, 1024, 16
DEPTH = D // H          # 64
HG = 8                  # heads per core
DQ = HG * DEPTH         # 512 head-dims per core
SCALE = float(D) ** -0.5
P = 128
KO = D // P             # 8 contraction chunks for the projections
NO = N // P             # 8 nk tiles
OQ = DQ // P            # 4 dq chunks


def _build(with_qb, with_kb, with_vb, with_mask):
    nc = bacc.Bacc("TRN2", target_bir_lowering=False, debug=False, num_devices=8)

    xT = nc.dram_tensor("xT", [D, N], F32R, kind="ExternalInput")
    lT = nc.dram_tensor("lT", [D, N], F32R, kind="ExternalInput")
    wqT = nc.dram_tensor("wqT", [D, DQ], F32R, kind="ExternalInput")
    wkT = nc.dram_tensor("wkT", [D, DQ], F32R, kind="ExternalInput")
    wvT = nc.dram_tensor("wvT", [D, DQ], F32R, kind="ExternalInput")
    woT = nc.dram_tensor("woT", [DQ, D], F32R, kind="ExternalInput")
    if with_qb:
        bq = nc.dram_tensor("bq", [P, OQ], F32, kind="ExternalInput")
    if with_kb:
        bk = nc.dram_tensor("bk", [P, OQ], F32, kind="ExternalInput")
    if with_vb:
        bv = nc.dram_tensor("bv", [P, OQ], F32, kind="ExternalInput")
    if with_mask:
        maskTf = nc.dram_tensor("maskTf", [N, N], F32R, kind="ExternalInput")

    e_un = nc.dram_tensor("e_un", [HG, N, N], F32R, kind="ExternalOutput")
    zeta = nc.dram_tensor("zeta", [HG, N], F32, kind="ExternalOutput")
    o1t = nc.dram_tensor("o1t", [DQ, N], F32R, kind="ExternalOutput")
    outp = nc.dram_tensor("outp", [N, D], F32, kind="ExternalOutput")

    xT_r = xT.rearrange("(ko p) n -> p ko n", p=P)
    lT_r = lT.rearrange("(ko p) n -> p ko n", p=P)
    wqT_r = wqT.rearrange("(ko p) dq -> p ko dq", p=P)
    wkT_r = wkT.rearrange("(ko p) dq -> p ko dq", p=P)
    wvT_r = wvT.rearrange("(ko p) dq -> p ko dq", p=P)
    woT_r = woT.rearrange("(ko p) d -> p ko d", p=P)
    o1t_r = o1t.rearrange("(o p) n -> p o n", p=P)

    with tile.TileContext(nc) as tc:
        with (
            tc.tile_pool(name="big", bufs=3 if with_mask else 2) as pool_big,
            tc.tile_pool(name="w", bufs=3) as pool_w,
            tc.tile_pool(name="persist", bufs=1) as pool_p,
            tc.tile_pool(name="small", bufs=2) as pool_s,
            tc.tile_pool(name="outsb", bufs=3) as pool_o,
            tc.tile_pool(name="ps", bufs=4, space="PSUM") as ps,
            tc.tile_pool(name="ps_av", bufs=2, space="PSUM") as ps_av,
            tc.tile_pool(name="ps_rb", bufs=2, space="PSUM") as ps_rb,
        ):
            # ---- Phase A: load inputs -------------------------------------
            xT_sb = pool_big.tile([P, KO, N], F32R, tag="big")
            lT_sb = pool_big.tile([P, KO, N], F32R, tag="big")
            nc.sync.dma_start(xT_sb[:], xT_r[:])
            nc.sync.dma_start(lT_sb[:], lT_r[:])
            wq_sb = pool_w.tile([P, KO, DQ], F32R, tag="w")
            wk_sb = pool_w.tile([P, KO, DQ], F32R, tag="w")
            wv_sb = pool_w.tile([P, KO, DQ], F32R, tag="w")
            nc.sync.dma_start(wq_sb[:], wqT_r[:])
            nc.sync.dma_start(wk_sb[:], wkT_r[:])
            nc.sync.dma_start(wv_sb[:], wvT_r[:])
            if with_mask:
                mk_sb = pool_big.tile([P, NO, N], F32R, tag="big")
                nc.sync.dma_start(mk_sb[:], maskTf.rearrange("(o p) n -> p o n", p=P))
            if with_qb:
                bq_sb = pool_s.tile([P, OQ], F32, tag="bias")
                nc.sync.dma_start(bq_sb[:], bq[:])
            if with_kb:
                bk_sb = pool_s.tile([P, OQ], F32, tag="bias")
                nc.sync.dma_start(bk_sb[:], bk[:])
            if with_vb:
                bv_sb = pool_s.tile([P, OQ], F32, tag="bias")
                nc.sync.dma_start(bv_sb[:], bv[:])

            # ---- Phase B: QKV projections ---------------------------------
            QT_sb = pool_p.tile([P, OQ, N], F32R, tag="qt")
            KT_sb = pool_p.tile([P, OQ, N], F32R, tag="kt")
            # V' layout: [nk_part, nk_outer, head, 66] -- 64 V cols + ones + pad
            V_sb = pool_p.tile([P, NO, HG, 66], F32R, tag="v")
            nc.vector.tensor_copy(
                V_sb[:, :, :, 64:65], nc.const_aps.tensor(1.0, (P, NO, HG, 1), F32)
            )
            nc.vector.tensor_copy(
                V_sb[:, :, :, 65:66], nc.const_aps.tensor(0.0, (P, NO, HG, 1), F32)
            )

            for proj, w_sb, dst, bias in (
                ("q", wq_sb, QT_sb, bq_sb if with_qb else None),
                ("k", wk_sb, KT_sb, bk_sb if with_kb else None),
            ):
                src_sb = xT_sb if proj == "q" else lT_sb
                for o in range(OQ):
                    for nh in range(2):
                        pt = ps.tile([P, 512], F32, tag="ps")
                        for ko in range(KO):
                            nc.tensor.matmul(
                                pt[:],
                                w_sb[:, ko, o * P:(o + 1) * P],
                                src_sb[:, ko, nh * 512:(nh + 1) * 512],
                                start=(ko == 0),
                                stop=(ko == KO - 1),
                            )
                        dslc = dst[:, o, nh * 512:(nh + 1) * 512]
                        if bias is not None:
                            nc.vector.tensor_scalar(
                                dslc, pt[:], bias[:, o:o + 1], None, OP.add
                            )
                        else:
                            nc.vector.tensor_copy(dslc, pt[:])

            for no in range(NO):
                pt = ps.tile([P, 512], F32, tag="ps")
                for ko in range(KO):
                    nc.tensor.matmul(
                        pt[:],
                        lT_sb[:, ko, no * P:(no + 1) * P],
                        wv_sb[:, ko, :],
                        start=(ko == 0),
                        stop=(ko == KO - 1),
                    )
                # scatter [128, 512] -> per-head 64-col groups, stride 66
                nc.vector.tensor_copy(
                    V_sb[:, no, :, 0:64],
                    pt[:].rearrange("p (h d) -> p h d", h=HG),
                )

            # ---- Phase C: attention per head ------------------------------
            ones_sb = pool_p.tile([1, 64], F32R, tag="ones")
            nc.vector.tensor_copy(ones_sb[:], nc.const_aps.tensor(1.0, (1, 64), F32))
            O_sb = pool_p.tile([P, OQ, N], F32R, tag="o")

            for h in range(HG):
                p0 = (h % 2) * 64
                oq = h // 2
                E_t = pool_big.tile([P, NO, N], F32R, tag="big")
                for no in range(NO):
                    for nh in range(2):
                        s = slice(nh * 512, (nh + 1) * 512)
                        pt = ps.tile([P, 512], F32, tag="ps")
                        nc.tensor.matmul(
                            pt[:],
                            KT_sb[p0:p0 + 64, oq, no * P:(no + 1) * P],
                            QT_sb[p0:p0 + 64, oq, s],
                            start=True,
                            stop=True,
                        )
                        nc.scalar.activation(E_t[:, no, s], pt[:], AF.Exp, scale=SCALE)
                        if with_mask:
                            nc.vector.tensor_tensor(
                                E_t[:, no, s], E_t[:, no, s], mk_sb[:, no, s], OP.mult
                            )
                    nc.sync.dma_start(e_un[h, no * P:(no + 1) * P, :], E_t[:, no, :])

                R_t = pool_s.tile([1, N], F32R, tag="r")
                Rb_t = pool_s.tile([64, N], F32, tag="rb")
                z_t = pool_s.tile([1, N], F32, tag="z")
                for nh in range(2):
                    s = slice(nh * 512, (nh + 1) * 512)
                    po = ps_av.tile([66, 512], F32, tag="ps_av")
                    for no in range(NO):
                        nc.tensor.matmul(
                            po[:],
                            V_sb[:, no, h, :],
                            E_t[:, no, s],
                            start=(no == 0),
                            stop=(no == NO - 1),
                        )
                    nc.vector.tensor_copy(z_t[0:1, s], po[64:65, :])
                    with nc.allow_low_precision(
                        reason="1/Z rounded to fp32r as PE-broadcast operand"
                    ):
                        nc.vector.reciprocal(R_t[0:1, s], po[64:65, :])
                    prb = ps_rb.tile([64, 512], F32, tag="ps_rb")
                    nc.tensor.matmul(prb[:], ones_sb[:], R_t[0:1, s], start=True, stop=True)
                    nc.vector.tensor_copy(Rb_t[:, s], prb[:])
                    oslc = O_sb[p0:p0 + 64, oq, s]
                    nc.vector.tensor_tensor(oslc, po[0:64, :], Rb_t[:, s], OP.mult)
                    if with_vb:
                        nc.vector.tensor_scalar(
                            oslc, oslc, bv_sb[p0:p0 + 64, oq:oq + 1], None, OP.add
                        )
                nc.sync.dma_start(zeta[h][None, :], z_t[:])

            nc.sync.dma_start(o1t_r[:], O_sb[:])

            # ---- Phase D: output projection -------------------------------
            wo_sb = pool_w.tile([P, OQ, D], F32R, tag="w")
            nc.sync.dma_start(wo_sb[:], woT_r[:])
            for m in range(NO):
                for jh in range(2):
                    s = slice(jh * 512, (jh + 1) * 512)
                    pt = ps.tile([P, 512], F32, tag="ps")
                    for ko in range(OQ):
                        nc.tensor.matmul(
                            pt[:],
                            O_sb[:, ko, m * P:(m + 1) * P],
                            wo_sb[:, ko, s],
                            start=(ko == 0),
                            stop=(ko == OQ - 1),
                        )
                    ot = pool_o.tile([P, 512], F32, tag="osb")
                    nc.vector.tensor_copy(ot[:], pt[:])
                    nc.sync.dma_start(outp[m * P:(m + 1) * P, s], ot[:])

    nc.finalize()
    return nc


_NC_CACHE = {}


def _get_nc(flags):
    if flags not in _NC_CACHE:
        _NC_CACHE[flags] = _build(*flags)
    return _NC_CACHE[flags]


def kernel(V_token, L_token, pad_mask, Wq_w, Wq_b, Wk_w, Wk_b, Wv_w, Wv_b,
           Wo_w, Wo_b, **_):
    V_token = np.asarray(V_token, dtype=np.float32)
    L_token = np.asarray(L_token, dtype=np.float32)
    pad_mask = np.asarray(pad_mask)
    Wq_w = np.asarray(Wq_w, dtype=np.float32)
    Wk_w = np.asarray(Wk_w, dtype=np.float32)
    Wv_w = np.asarray(Wv_w, dtype=np.float32)
    Wo_w = np.asarray(Wo_w, dtype=np.float32)
    Wq_b = np.asarray(Wq_b, dtype=np.float32)
    Wk_b = np.asarray(Wk_b, dtype=np.float32)
    Wv_b = np.asarray(Wv_b, dtype=np.float32)
    Wo_b = np.asarray(Wo_b, dtype=np.float32)

    with_qb = bool(np.any(Wq_b))
    with_kb = bool(np.any(Wk_b))
    with_vb = bool(np.any(Wv_b))
    with_mask = bool(np.any(pad_mask))
    nc = _get_nc((with_qb, with_kb, with_vb, with_mask))

    in_maps = []
    for c in range(8):
        b, g = c // 2, c % 2
        hs = slice(g * DQ, (g + 1) * DQ)
        m = {
            "xT": np.ascontiguousarray(V_token[b].T),
            "lT": np.ascontiguousarray(L_token[b].T),
            "wqT": np.ascontiguousarray(Wq_w[hs].T),
            "wkT": np.ascontiguousarray(Wk_w[hs].T),
            "wvT": np.ascontiguousarray(Wv_w[hs].T),
            "woT": np.ascontiguousarray(Wo_w[:, hs].T),
        }
        if with_qb:
            m["bq"] = np.ascontiguousarray(Wq_b[hs].reshape(OQ, P).T)
        if with_kb:
            m["bk"] = np.ascontiguousarray(Wk_b[hs].reshape(OQ, P).T)
        if with_vb:
            m["bv"] = np.ascontiguousarray(Wv_b[hs].reshape(OQ, P).T)
        if with_mask:
            m["maskTf"] = np.ascontiguousarray(
                1.0 - pad_mask[b].T.astype(np.float32)
            )
        in_maps.append(m)

    res = run_bass_kernel_spmd(nc, in_maps, list(range(8)))

    att = np.empty((B, H, N, N), dtype=np.float32)
    out_1 = np.empty((B, H, N, DEPTH), dtype=np.float32)
    out = np.zeros((B, N, D), dtype=np.float32)
    for c in range(8):
        b, g = c // 2, c % 2
        r = res.results[c]
        e = r["e_un"]          # [HG, nk, nq]
        z = r["zeta"]          # [HG, nq]
        o1 = r["o1t"]          # [DQ, nq]
        for h in range(HG):
            Hh = g * HG + h
            att[b, Hh] = (e[h] / z[h][None, :]).T
        out_1[b, g * HG:(g + 1) * HG] = (
            o1.reshape(HG, DEPTH, N).transpose(0, 2, 1)
        )
        out[b] += r["outp"]
    out += Wo_b[None, None, :]
    return (out_1, out, att)


# revision 16
# speedup vs baseline: 1.2096x; 1.2096x over previous
"""Trainium2 Bass kernel for cross multi-head attention (B=4, N=1024, D=1024, H=16).

Sharding: 8 cores = 4 batches x 2 head-groups (8 heads each), Megatron-style.
Each core computes, for its (batch b, head-group g):
  Q^T, K^T = W[hs] @ x^T / l^T      (projections, transposed layout)
  S^T_h    = K_h @ Q_h^T            (scores, [nk, nq] layout)
  E        = exp(S^T * scale)       (unnormalized; softmax max-shift is
                                     unnecessary: |scores| <~ 1)
  O'^T     = V'^T E  with V' = [V | 1]  -> rows 0:64 = unnorm AV, row 64 = Z
  O^T      = O'^T[0:64] * (1/Z)     (normalized attention output, transposed)
  outp     = O^T.T @ Wo[:, hs].T    (partial out-projection)
Host side: gathers per-core outputs, transposes/normalizes att and out_1,
sums the two head-group partials of out, adds biases that commute
(Wo bias). All matmuls run as float32r (tf32-like, ~1.5e-4 rel err).
"""

import sys

if "/opt/trn_rl_repo" not in sys.path:
    sys.path.insert(0, "/opt/trn_rl_repo")

import numpy as np

import concourse.bacc as bacc
import concourse.mybir as mybir
import concourse.tile as tile
from concourse.bass_utils import run_bass_kernel_spmd

F32 = mybir.dt.float32
F32R = mybir.dt.float32r
AF = mybir.ActivationFunctionType
OP = mybir.AluOpType

B, N, D, H = 4, 1024, 1024, 16
DEPTH = D // H          # 64
HG = 8                  # heads per core
DQ = HG * DEPTH         # 512 head-dims per core
SCALE = float(D) ** -0.5
P = 128
KO = D // P             # 8 contraction chunks for the projections
NO = N // P             # 8 nk tiles
OQ = DQ // P            # 4 dq chunks


def _build(with_qb, with_kb, with_vb, with_mask):
    nc = bacc.Bacc("TRN2", target_bir_lowering=False, debug=False, num_devices=8)

    xT = nc.dram_tensor("xT", [D, N], F32R, kind="ExternalInput")
    lT = nc.dram_tensor("lT", [D, N], F32R, kind="ExternalInput")
    wqT = nc.dram_tensor("wqT", [D, DQ], F32R, kind="ExternalInput")
    wkT = nc.dram_tensor("wkT", [D, DQ], F32R, kind="ExternalInput")
    wvT = nc.dram_tensor("wvT", [D, DQ], F32R, kind="ExternalInput")
    woT = nc.dram_tensor("woT", [DQ, D], F32R, kind="ExternalInput")
    if with_qb:
        bq = nc.dram_tensor("bq", [P, OQ], F32, kind="ExternalInput")
    if with_kb:
        bk = nc.dram_tensor("bk", [P, OQ], F32, kind="ExternalInput")
    if with_vb:
        bv = nc.dram_tensor("bv", [P, OQ], F32, kind="ExternalInput")
    if with_mask:
        maskTf = nc.dram_tensor("maskTf", [N, N], F32R, kind="ExternalInput")

    e_un = nc.dram_tensor("e_un", [HG, N, N], F32R, kind="ExternalOutput")
    zeta = nc.dram_tensor("zeta", [HG, N], F32R, kind="ExternalOutput")
    o1t = nc.dram_tensor("o1t", [DQ, N], F32R, kind="ExternalOutput")
    outp = nc.dram_tensor("outp", [N, D], F32, kind="ExternalOutput")

    xT_r = xT.rearrange("(ko p) n -> p ko n", p=P)
    lT_r = lT.rearrange("(ko p) n -> p ko n", p=P)
    wqT_r = wqT.rearrange("(ko p) dq -> p ko dq", p=P)
    wkT_r = wkT.rearrange("(ko p) dq -> p ko dq", p=P)
    wvT_r = wvT.rearrange("(ko p) dq -> p ko dq", p=P)
    woT_r = woT.rearrange("(ko p) d -> p ko d", p=P)
    o1t_r = o1t.rearrange("(o p) n -> p o n", p=P)

    with tile.TileContext(nc) as tc:
        with (
            tc.tile_pool(name="big", bufs=3 if with_mask else 2) as pool_big,
            tc.tile_pool(name="w", bufs=3) as pool_w,
            tc.tile_pool(name="persist", bufs=1) as pool_p,
            tc.tile_pool(name="small", bufs=2) as pool_s,
            tc.tile_pool(name="outsb", bufs=3) as pool_o,
            tc.tile_pool(name="ps", bufs=4, space="PSUM") as ps,
            tc.tile_pool(name="ps_av", bufs=2, space="PSUM") as ps_av,
            tc.tile_pool(name="ps_rb", bufs=2, space="PSUM") as ps_rb,
        ):
            # ---- Phase A: load inputs -------------------------------------
            xT_sb = pool_big.tile([P, KO, N], F32R, tag="big")
            lT_sb = pool_big.tile([P, KO, N], F32R, tag="big")
            nc.sync.dma_start(xT_sb[:], xT_r[:])
            nc.sync.dma_start(lT_sb[:], lT_r[:])
            wq_sb = pool_w.tile([P, KO, DQ], F32R, tag="w")
            wk_sb = pool_w.tile([P, KO, DQ], F32R, tag="w")
            wv_sb = pool_w.tile([P, KO, DQ], F32R, tag="w")
            nc.sync.dma_start(wq_sb[:], wqT_r[:])
            nc.sync.dma_start(wk_sb[:], wkT_r[:])
            nc.sync.dma_start(wv_sb[:], wvT_r[:])
            if with_mask:
                mk_sb = pool_big.tile([P, NO, N], F32R, tag="big")
                nc.sync.dma_start(mk_sb[:], maskTf.rearrange("(o p) n -> p o n", p=P))
            if with_qb:
                bq_sb = pool_s.tile([P, OQ], F32, tag="bias")
                nc.sync.dma_start(bq_sb[:], bq[:])
            if with_kb:
                bk_sb = pool_s.tile([P, OQ], F32, tag="bias")
                nc.sync.dma_start(bk_sb[:], bk[:])
            if with_vb:
                bv_sb = pool_s.tile([P, OQ], F32, tag="bias")
                nc.sync.dma_start(bv_sb[:], bv[:])

            # ---- Phase B: QKV projections ---------------------------------
            QT_sb = pool_p.tile([P, OQ, N], F32R, tag="qt")
            KT_sb = pool_p.tile([P, OQ, N], F32R, tag="kt")
            # V' layout: [nk_part, nk_outer, head, 66] -- 64 V cols + ones + pad
            V_sb = pool_p.tile([P, NO, HG, 66], F32R, tag="v")
            nc.vector.tensor_copy(
                V_sb[:, :, :, 64:65], nc.const_aps.tensor(1.0, (P, NO, HG, 1), F32)
            )
            nc.vector.tensor_copy(
                V_sb[:, :, :, 65:66], nc.const_aps.tensor(0.0, (P, NO, HG, 1), F32)
            )

            for proj, w_sb, dst, bias in (
                ("q", wq_sb, QT_sb, bq_sb if with_qb else None),
                ("k", wk_sb, KT_sb, bk_sb if with_kb else None),
            ):
                src_sb = xT_sb if proj == "q" else lT_sb
                for o in range(OQ):
                    for nh in range(2):
                        pt = ps.tile([P, 512], F32, tag="ps")
                        for ko in range(KO):
                            nc.tensor.matmul(
                                pt[:],
                                w_sb[:, ko, o * P:(o + 1) * P],
                                src_sb[:, ko, nh * 512:(nh + 1) * 512],
                                start=(ko == 0),
                                stop=(ko == KO - 1),
                            )
                        dslc = dst[:, o, nh * 512:(nh + 1) * 512]
                        if bias is not None:
                            nc.vector.tensor_scalar(
                                dslc, pt[:], bias[:, o:o + 1], None, OP.add
                            )
                        else:
                            nc.vector.tensor_copy(dslc, pt[:])

            for no in range(NO):
                pt = ps.tile([P, 512], F32, tag="ps")
                for ko in range(KO):
                    nc.tensor.matmul(
                        pt[:],
                        lT_sb[:, ko, no * P:(no + 1) * P],
                        wv_sb[:, ko, :],
                        start=(ko == 0),
                        stop=(ko == KO - 1),
                    )
                # scatter [128, 512] -> per-head 64-col groups, stride 66
                nc.vector.tensor_copy(
                    V_sb[:, no, :, 0:64],
                    pt[:].rearrange("p (h d) -> p h d", h=HG),
                )

            # ---- Phase C: attention per head ------------------------------
            ones_sb = pool_p.tile([1, 64], F32R, tag="ones")
            nc.vector.tensor_copy(ones_sb[:], nc.const_aps.tensor(1.0, (1, 64), F32))
            O_sb = pool_p.tile([P, OQ, N], F32R, tag="o")

            def emit_scores(h, E_t):
                p0 = (h % 2) * 64
                oq = h // 2
                for no in range(NO):
                    for nh in range(2):
                        s = slice(nh * 512, (nh + 1) * 512)
                        pt = ps.tile([P, 512], F32, tag="ps")
                        nc.tensor.matmul(
                            pt[:],
                            KT_sb[p0:p0 + 64, oq, no * P:(no + 1) * P],
                            QT_sb[p0:p0 + 64, oq, s],
                            start=True,
                            stop=True,
                        )
                        nc.scalar.activation(E_t[:, no, s], pt[:], AF.Exp, scale=SCALE)
                        if with_mask:
                            nc.vector.tensor_tensor(
                                E_t[:, no, s], E_t[:, no, s], mk_sb[:, no, s], OP.mult
                            )
                nc.sync.dma_start(
                    e_un[h].rearrange("(no p) n -> p no n", p=P), E_t[:]
                )

            def emit_av(h, E_t):
                p0 = (h % 2) * 64
                oq = h // 2
                Rb_t = pool_s.tile([64, N], F32, tag="rb")
                z_t = pool_s.tile([1, N], F32R, tag="z")
                for nh in range(2):
                    s = slice(nh * 512, (nh + 1) * 512)
                    po = ps_av.tile([66, 512], F32, tag="ps_av")
                    for no in range(NO):
                        nc.tensor.matmul(
                            po[:],
                            V_sb[:, no, h, :],
                            E_t[:, no, s],
                            start=(no == 0),
                            stop=(no == NO - 1),
                        )
                    nc.vector.tensor_copy(z_t[0:1, s], po[64:65, :])
                    pzb = ps_rb.tile([64, 512], F32, tag="ps_rb")
                    nc.tensor.matmul(
                        pzb[:], ones_sb[:], z_t[0:1, s], start=True, stop=True
                    )
                    nc.vector.reciprocal_approx_fast(Rb_t[:, s], pzb[:])
                    oslc = O_sb[p0:p0 + 64, oq, s]
                    nc.vector.tensor_tensor(oslc, po[0:64, :], Rb_t[:, s], OP.mult)
                    if with_vb:
                        nc.vector.tensor_scalar(
                            oslc, oslc, bv_sb[p0:p0 + 64, oq:oq + 1], None, OP.add
                        )
                nc.sync.dma_start(zeta[h][None, :], z_t[:])

            E_ts = {}
            for h in range(HG):
                E_ts[h] = pool_big.tile([P, NO, N], F32R, tag="big", name=f"E_{h}")
                emit_scores(h, E_ts[h])
                if h > 0:
                    emit_av(h - 1, E_ts[h - 1])
            emit_av(HG - 1, E_ts[HG - 1])

            nc.sync.dma_start(o1t_r[:], O_sb[:])

            # ---- Phase D: output projection -------------------------------
            wo_sb = pool_w.tile([P, OQ, D], F32R, tag="w")
            nc.sync.dma_start(wo_sb[:], woT_r[:])
            for m in range(NO):
                for jh in range(2):
                    s = slice(jh * 512, (jh + 1) * 512)
                    pt = ps.tile([P, 512], F32, tag="ps")
                    for ko in range(OQ):
                        nc.tensor.matmul(
                            pt[:],
                            O_sb[:, ko, m * P:(m + 1) * P],
                            wo_sb[:, ko, s],
                            start=(ko == 0),
                            stop=(ko == OQ - 1),
                        )
                    ot = pool_o.tile([P, 512], F32, tag="osb")
                    nc.vector.tensor_copy(ot[:], pt[:])
                    nc.sync.dma_start(outp[m * P:(m + 1) * P, s], ot[:])

    nc.finalize()
    return nc


_NC_CACHE = {}


def _get_nc(flags):
    if flags not in _NC_CACHE:
        _NC_CACHE[flags] = _build(*flags)
    return _NC_CACHE[flags]


def kernel(V_token, L_token, pad_mask, Wq_w, Wq_b, Wk_w, Wk_b, Wv_w, Wv_b,
           Wo_w, Wo_b, **_):
    V_token = np.asarray(V_token, dtype=np.float32)
    L_token = np.asarray(L_token, dtype=np.float32)
    pad_mask = np.asarray(pad_mask)
    Wq_w = np.asarray(Wq_w, dtype=np.float32)
    Wk_w = np.asarray(Wk_w, dtype=np.float32)
    Wv_w = np.asarray(Wv_w, dtype=np.float32)
    Wo_w = np.asarray(Wo_w, dtype=np.float32)
    Wq_b = np.asarray(Wq_b, dtype=np.float32)
    Wk_b = np.asarray(Wk_b, dtype=np.float32)
    Wv_b = np.asarray(Wv_b, dtype=np.float32)
    Wo_b = np.asarray(Wo_b, dtype=np.float32)

    with_qb = bool(np.any(Wq_b))
    with_kb = bool(np.any(Wk_b))
    with_vb = bool(np.any(Wv_b))
    with_mask = bool(np.any(pad_mask))
    nc = _get_nc((with_qb, with_kb, with_vb, with_mask))

    in_maps = []
    for c in range(8):
        b, g = c // 2, c % 2
        hs = slice(g * DQ, (g + 1) * DQ)
        m = {
            "xT": np.ascontiguousarray(V_token[b].T),
            "lT": np.ascontiguousarray(L_token[b].T),
            "wqT": np.ascontiguousarray(Wq_w[hs].T),
            "wkT": np.ascontiguousarray(Wk_w[hs].T),
            "wvT": np.ascontiguousarray(Wv_w[hs].T),
            "woT": np.ascontiguousarray(Wo_w[:, hs].T),
        }
        if with_qb:
            m["bq"] = np.ascontiguousarray(Wq_b[hs].reshape(OQ, P).T)
        if with_kb:
            m["bk"] = np.ascontiguousarray(Wk_b[hs].reshape(OQ, P).T)
        if with_vb:
            m["bv"] = np.ascontiguousarray(Wv_b[hs].reshape(OQ, P).T)
        if with_mask:
            m["maskTf"] = np.ascontiguousarray(
                1.0 - pad_mask[b].T.astype(np.float32)
            )
        in_maps.append(m)

    res = run_bass_kernel_spmd(nc, in_maps, list(range(8)))

    att = np.empty((B, H, N, N), dtype=np.float32)
    out_1 = np.empty((B, H, N, DEPTH), dtype=np.float32)
    out = np.zeros((B, N, D), dtype=np.float32)
    for c in range(8):
        b, g = c // 2, c % 2
        r = res.results[c]
        e = r["e_un"]          # [HG, nk, nq]
        z = r["zeta"]          # [HG, nq]
        o1 = r["o1t"]          # [DQ, nq]
        for h in range(HG):
            Hh = g * HG + h
            att[b, Hh] = (e[h] / z[h][None, :]).T
        out_1[b, g * HG:(g + 1) * HG] = (
            o1.reshape(HG, DEPTH, N).transpose(0, 2, 1)
        )
        out[b] += r["outp"]
    out += Wo_b[None, None, :]
    return (out_1, out, att)
